# revision 3
# baseline (speedup 1.0000x reference)
"""MoE layer (E=8 experts, top-2 routing) on 8 Trainium2 NeuronCores.

Strategy: expert-parallel. The host computes the (tiny) gating network in
fp64 -- logits = x @ wg + bg, top-2, softmax -- and dispatches each token to
the cores owning its two selected experts (the "all-to-all dispatch tokens
by routing decision" sharding). Each core runs its expert's FFN
    y = relu(x_e @ w1[e] + b1[e]) @ w2[e]
over its gathered tokens (padded to a common static NT), scales rows by the
gate weight on-device, and the host scatter-adds the two slots per token
back together (plus the combine@b2 bias term).

Hardcoded problem shape: x [4,4096,512], w1 [8,512,1024], w2 [8,1024,512],
wg [512,8], top_k=2.
"""

import os
import numpy as np

B, S, D, F, E = 4, 4096, 512, 1024, 8
TOP_K = 2
N_CORES = 8

# matmul operand dtype mode: "bf16" | "f32" | "f32r"
DT_MODE = os.environ.get("MOE_DT", "bf16")
TRACE = os.environ.get("MOE_TRACE", "0") == "1"


def _build_program(NT, mode):
    from concourse import bacc, tile, mybir

    dt = mybir.dt
    DT = {"bf16": dt.bfloat16, "f32": dt.float32, "f32r": dt.float32r}[mode]

    def mm(ap):
        return ap

    nc = bacc.Bacc("TRN2", target_bir_lowering=False, debug=False)

    xt_d = nc.dram_tensor("xt", [D, NT], DT, kind="ExternalInput").ap()
    w1_d = nc.dram_tensor("w1t", [D, F], DT, kind="ExternalInput").ap()
    w2_d = nc.dram_tensor("w2t", [F, D], DT, kind="ExternalInput").ap()
    b1_d = nc.dram_tensor("b1c", [128, F // 128], dt.float32, kind="ExternalInput").ap()
    g_d = nc.dram_tensor("gate2", [128, NT // 128], dt.float32, kind="ExternalInput").ap()
    y_d = nc.dram_tensor("y", [NT, D], dt.float32, kind="ExternalOutput").ap()

    KD = D // 128   # 4 contraction blocks for mm1
    FB = F // 128   # 8 F blocks (partition blocks of h / contraction blocks of mm2)
    NCHUNK = NT // 512

    with tile.TileContext(nc) as tc:
        with (
            tc.tile_pool(name="w", bufs=1) as wpool,
            tc.tile_pool(name="x", bufs=3) as xpool,
            tc.tile_pool(name="h", bufs=2) as hpool,
            tc.tile_pool(name="o", bufs=4) as opool,
            tc.tile_pool(name="ps1", bufs=4, space="PSUM") as ps1,
            tc.tile_pool(name="ps2", bufs=4, space="PSUM") as ps2,
        ):
            w1_sb = wpool.tile([128, KD, F], DT)
            for kc in range(KD):
                nc.sync.dma_start(out=w1_sb[:, kc, :], in_=w1_d[kc * 128:(kc + 1) * 128, :])
            w2_sb = wpool.tile([128, FB, D], DT)
            for fb in range(FB):
                nc.sync.dma_start(out=w2_sb[:, fb, :], in_=w2_d[fb * 128:(fb + 1) * 128, :])
            b1_sb = wpool.tile([128, FB], dt.float32)
            nc.sync.dma_start(out=b1_sb[:], in_=b1_d[:])
            g_sb = wpool.tile([128, NT // 128], dt.float32)
            nc.sync.dma_start(out=g_sb[:], in_=g_d[:])

            for t in range(NCHUNK):
                x_sb = xpool.tile([128, KD, 512], DT)
                for kc in range(KD):
                    nc.sync.dma_start(
                        out=x_sb[:, kc, :],
                        in_=xt_d[kc * 128:(kc + 1) * 128, t * 512:(t + 1) * 512],
                    )
                h_sb = hpool.tile([128, FB, 512], DT)
                for fb in range(FB):
                    p = ps1.tile([128, 512], dt.float32)
                    for kc in range(KD):
                        nc.tensor.matmul(
                            p[:],
                            mm(w1_sb[:, kc, fb * 128:(fb + 1) * 128]),
                            mm(x_sb[:, kc, :]),
                            start=(kc == 0),
                            stop=(kc == KD - 1),
                        )
                    nc.scalar.activation(
                        h_sb[:, fb, :],
                        p[:],
                        mybir.ActivationFunctionType.Relu,
                        bias=b1_sb[:, fb:fb + 1],
                        scale=1.0,
                    )
                for tb in range(4):
                    p2 = ps2.tile([128, 512], dt.float32)
                    for fb in range(FB):
                        nc.tensor.matmul(
                            p2[:],
                            mm(h_sb[:, fb, tb * 128:(tb + 1) * 128]),
                            mm(w2_sb[:, fb, :]),
                            start=(fb == 0),
                            stop=(fb == FB - 1),
                        )
                    o_sb = opool.tile([128, 512], dt.float32)
                    nc.vector.tensor_scalar_mul(
                        o_sb[:], p2[:], g_sb[:, t * 4 + tb:t * 4 + tb + 1]
                    )
                    nc.sync.dma_start(
                        out=y_d[t * 512 + tb * 128:t * 512 + (tb + 1) * 128, :],
                        in_=o_sb[:],
                    )
    nc.compile()
    return nc


def _install_ntff_hook():
    """Register the axon NTFF profiling hook that run_bass_kernel_spmd
    (trace=True) looks for under antenv.axon_hooks; this container's antenv
    lacks that module, so recreate it via ctypes against libaxon_pjrt.so."""
    import sys, types, ctypes, contextlib

    if "antenv.axon_hooks" in sys.modules:
        return
    try:
        lib = ctypes.CDLL("/opt/axon/libaxon_pjrt.so")
    except OSError:
        return
    if not hasattr(lib, "axon_start_nrt_profile"):
        return
    lib.axon_start_nrt_profile.argtypes = [ctypes.POINTER(ctypes.c_int64), ctypes.c_size_t]
    lib.axon_start_nrt_profile.restype = ctypes.c_int64
    lib.axon_stop_nrt_profile.argtypes = [ctypes.c_char_p]
    lib.axon_stop_nrt_profile.restype = ctypes.c_int64

    @contextlib.contextmanager
    def _hook(output_dir, device_ids):
        import jax

        jax.devices()
        if device_ids:
            ids = (ctypes.c_int64 * len(device_ids))(*device_ids)
            rc = lib.axon_start_nrt_profile(ids, len(device_ids))
        else:
            rc = lib.axon_start_nrt_profile(None, 0)
        if rc != 0:
            raise RuntimeError(f"axon_start_nrt_profile rc={rc}")
        try:
            yield
        finally:
            n = lib.axon_stop_nrt_profile(str(output_dir).encode())
            print(f"profile: {n} ntff file(s) written to {output_dir}")

    mod = types.ModuleType("antenv.axon_hooks")
    _holder = {"h": _hook}
    mod.set_axon_ntff_profile_hook = lambda h: _holder.__setitem__("h", h)
    mod.get_axon_ntff_profile_hook = lambda: _holder["h"]
    sys.modules["antenv.axon_hooks"] = mod

    # avoid the S3/Fish artifact upload in the trace post-processing path
    import concourse.bass_utils as bu

    bu.upload_artifacts = lambda tmpdir: str(tmpdir)


def kernel(**inputs):
    from concourse.bass_utils import run_bass_kernel_spmd

    if TRACE:
        _install_ntff_hook()

    x = np.asarray(inputs["x"], np.float32)
    w1 = np.asarray(inputs["w1"], np.float32)
    b1 = np.asarray(inputs["b1"], np.float32)
    w2 = np.asarray(inputs["w2"], np.float32)
    b2 = np.asarray(inputs["b2"], np.float32)
    wg = np.asarray(inputs["wg"], np.float32)
    bg = np.asarray(inputs["bg"], np.float32)

    T = x.shape[0] * x.shape[1]
    xf = x.reshape(T, D)

    # ---- host gating (fp64): logits -> top-2 (jax.lax.top_k tie order:
    # lower index wins -> stable argsort on -logits) -> softmax over top-2.
    logits = xf.astype(np.float64) @ wg.astype(np.float64) + bg.astype(np.float64)
    order = np.argsort(-logits, axis=1, kind="stable")
    top_idx = order[:, :TOP_K]                      # [T, K]
    top_vals = np.take_along_axis(logits, top_idx, axis=1)
    gwts = np.exp(top_vals - top_vals.max(axis=1, keepdims=True))
    gwts = gwts / gwts.sum(axis=1, keepdims=True)   # [T, K]

    # ---- dispatch: sort slots (t, k) by expert; per-expert contiguous runs.
    flat_expert = top_idx.ravel()                   # slot s = t*K + k
    perm = np.argsort(flat_expert, kind="stable")   # slots grouped by expert
    counts = np.bincount(flat_expert, minlength=E)
    cum = np.concatenate([[0], np.cumsum(counts)])
    slot_tok = perm // TOP_K                        # token of each sorted slot
    gates_sorted = gwts.ravel()[perm].astype(np.float32)

    NT = max(512, int(-(-counts.max() // 512)) * 512)
    NTG = NT // 128

    if DT_MODE == "bf16":
        import ml_dtypes
        io_dtype = ml_dtypes.bfloat16
    else:
        io_dtype = np.float32

    xf_io = xf.astype(io_dtype)
    w1_io = w1.astype(io_dtype)
    w2_io = w2.astype(io_dtype)

    in_maps = []
    for e in range(E):
        n = int(counts[e])
        toks = slot_tok[cum[e]:cum[e] + n]
        xt = np.zeros((D, NT), io_dtype)
        xt[:, :n] = xf_io[toks].T
        gate = np.zeros(NT, np.float32)
        gate[:n] = gates_sorted[cum[e]:cum[e] + n]
        in_maps.append({
            "xt": xt,
            "w1t": w1_io[e],
            "w2t": w2_io[e],
            "b1c": np.ascontiguousarray(b1[e].reshape(F // 128, 128).T),
            "gate2": np.ascontiguousarray(gate.reshape(NTG, 128).T),
        })

    nc = _build_program(NT, DT_MODE)
    res = run_bass_kernel_spmd(nc, in_maps, list(range(N_CORES)), trace=TRACE)
    if TRACE and res.exec_time_ns is not None:
        print(f"HW exec time: {res.exec_time_ns} ns")

    # ---- unshard: scatter slots back, sum the K slots per token, add b2 term.
    out_slots = np.zeros((T * TOP_K, D), np.float32)
    for e in range(E):
        n = int(counts[e])
        out_slots[perm[cum[e]:cum[e] + n]] = res.results[e]["y"][:n]
    out = out_slots.reshape(T, TOP_K, D).sum(axis=1)

    # combine @ b2 (gate-weighted expert output biases)
    combine = np.zeros((T, E), np.float32)
    np.put_along_axis(combine, top_idx, gwts.astype(np.float32), axis=1)
    out += combine @ b2

    return out.reshape(B, S, D).astype(np.float32)


# revision 6
# speedup vs baseline: 1.0150x; 1.0150x over previous
"""MoE layer (E=8 experts, top-2 routing) on 8 Trainium2 NeuronCores.

Strategy: expert-parallel. The host computes the (tiny) gating network in
fp64 -- logits = x @ wg + bg, top-2, softmax -- and dispatches each token to
the cores owning its two selected experts (the "all-to-all dispatch tokens
by routing decision" sharding). Each core runs its expert's FFN
    y = relu(x_e @ w1[e] + b1[e]) @ w2[e]
over its gathered tokens (padded to a common static NT), scales rows by the
gate weight on-device, and the host scatter-adds the two slots per token
back together (plus the combine@b2 bias term).

Hardcoded problem shape: x [4,4096,512], w1 [8,512,1024], w2 [8,1024,512],
wg [512,8], top_k=2.
"""

import os
import numpy as np

B, S, D, F, E = 4, 4096, 512, 1024, 8
TOP_K = 2
N_CORES = 8

# matmul operand dtype mode: "bf16" | "f32" | "f32r"
DT_MODE = os.environ.get("MOE_DT", "bf16")
TRACE = os.environ.get("MOE_TRACE", "0") == "1"


def _build_program(NT, mode):
    from concourse import bacc, tile, mybir
    from concourse.tile import add_dep_helper

    dt = mybir.dt
    DT = {"bf16": dt.bfloat16, "f32": dt.float32, "f32r": dt.float32r}[mode]

    nc = bacc.Bacc("TRN2", target_bir_lowering=False, debug=False)

    xt_d = nc.dram_tensor("xt", [D, NT], DT, kind="ExternalInput").ap()
    w1_d = nc.dram_tensor("w1t", [D, F], DT, kind="ExternalInput").ap()
    w2_d = nc.dram_tensor("w2t", [F, D], DT, kind="ExternalInput").ap()
    b1_d = nc.dram_tensor("b1c", [128, F // 128], dt.float32, kind="ExternalInput").ap()
    g_d = nc.dram_tensor("gate2", [128, NT // 128], dt.float32, kind="ExternalInput").ap()
    y_d = nc.dram_tensor("y", [NT, D], dt.float32, kind="ExternalOutput").ap()

    KD = D // 128   # 4 contraction blocks for mm1
    FB = F // 128   # 8 F blocks (partition blocks of h / contraction blocks of mm2)
    # chunk sizes along the token (moving) axis; remainder chunk last
    chunks = [512] * (NT // 512)
    if NT % 512:
        chunks.append(NT % 512)

    with tile.TileContext(nc) as tc:
        with (
            tc.tile_pool(name="w", bufs=1) as wpool,
            tc.tile_pool(name="x", bufs=3) as xpool,
            tc.tile_pool(name="h", bufs=2) as hpool,
            tc.tile_pool(name="o", bufs=4) as opool,
            tc.tile_pool(name="ps1", bufs=4, space="PSUM") as ps1,
            tc.tile_pool(name="ps2", bufs=4, space="PSUM") as ps2,
        ):
            # phase A loads: w1 + x chunk 0 (+ tiny b1/gate) — what mm1 of
            # chunk 0 needs. w2 and x chunk 1 are explicitly ordered after
            # these so the first matmuls don't wait on bandwidth-sharing
            # with loads that are only needed later.
            w1_sb = wpool.tile([128, KD, F], DT)
            for kc in range(KD):
                nc.sync.dma_start(out=w1_sb[:, kc, :], in_=w1_d[kc * 128:(kc + 1) * 128, :])
            b1_sb = wpool.tile([128, FB], dt.float32)
            nc.sync.dma_start(out=b1_sb[:], in_=b1_d[:])
            g_sb = wpool.tile([128, NT // 128], dt.float32)
            nc.sync.dma_start(out=g_sb[:], in_=g_d[:])

            x_tiles = {}
            x_dmas = {}

            def load_x(t):
                cs = chunks[t]
                off = 512 * t
                x_sb = xpool.tile([128, KD, cs], DT, tag="x")
                dmas = []
                for kc in range(KD):
                    dmas.append(nc.sync.dma_start(
                        out=x_sb[:, kc, :],
                        in_=xt_d[kc * 128:(kc + 1) * 128, off:off + cs],
                    ))
                x_tiles[t] = x_sb
                x_dmas[t] = dmas

            load_x(0)

            # phase B: w2, ordered after phase A's x0 loads
            w2_sb = wpool.tile([128, FB, D], DT)
            w2_dmas = []
            for fb in range(FB):
                i = nc.sync.dma_start(out=w2_sb[:, fb, :], in_=w2_d[fb * 128:(fb + 1) * 128, :])
                w2_dmas.append(i)
                add_dep_helper(x_dmas[0][-1].ins, i.ins, sync=True, reason="dma order: w2 after x0")

            for t in range(len(chunks)):
                cs = chunks[t]
                off = 512 * t
                if t + 1 < len(chunks):
                    load_x(t + 1)
                    if t == 0:
                        # x1 after w2 (later prefetches are throttled by
                        # the x pool slots)
                        for i in x_dmas[1]:
                            add_dep_helper(w2_dmas[-1].ins, i.ins, sync=True, reason="dma order: x1 after w2")
                x_sb = x_tiles.pop(t)
                h_sb = hpool.tile([128, FB, cs], DT, tag="h")
                for fb in range(FB):
                    p = ps1.tile([128, cs], dt.float32, tag="ps1")
                    for kc in range(KD):
                        nc.tensor.matmul(
                            p[:],
                            w1_sb[:, kc, fb * 128:(fb + 1) * 128],
                            x_sb[:, kc, :],
                            start=(kc == 0),
                            stop=(kc == KD - 1),
                        )
                    nc.scalar.activation(
                        h_sb[:, fb, :],
                        p[:],
                        mybir.ActivationFunctionType.Relu,
                        bias=b1_sb[:, fb:fb + 1],
                        scale=1.0,
                    )
                for tb in range(cs // 128):
                    p2 = ps2.tile([128, 512], dt.float32, tag="ps2")
                    for fb in range(FB):
                        nc.tensor.matmul(
                            p2[:],
                            h_sb[:, fb, tb * 128:(tb + 1) * 128],
                            w2_sb[:, fb, :],
                            start=(fb == 0),
                            stop=(fb == FB - 1),
                        )
                    o_sb = opool.tile([128, 512], dt.float32, tag="o")
                    nc.vector.tensor_scalar_mul(
                        o_sb[:], p2[:], g_sb[:, t * 4 + tb:t * 4 + tb + 1]
                    )
                    nc.sync.dma_start(
                        out=y_d[off + tb * 128:off + (tb + 1) * 128, :],
                        in_=o_sb[:],
                    )
    nc.compile()
    return nc


def _install_ntff_hook():
    """Register the axon NTFF profiling hook that run_bass_kernel_spmd
    (trace=True) looks for under antenv.axon_hooks; this container's antenv
    lacks that module, so recreate it via ctypes against libaxon_pjrt.so."""
    import sys, types, ctypes, contextlib

    if "antenv.axon_hooks" in sys.modules:
        return
    try:
        lib = ctypes.CDLL("/opt/axon/libaxon_pjrt.so")
    except OSError:
        return
    if not hasattr(lib, "axon_start_nrt_profile"):
        return
    lib.axon_start_nrt_profile.argtypes = [ctypes.POINTER(ctypes.c_int64), ctypes.c_size_t]
    lib.axon_start_nrt_profile.restype = ctypes.c_int64
    lib.axon_stop_nrt_profile.argtypes = [ctypes.c_char_p]
    lib.axon_stop_nrt_profile.restype = ctypes.c_int64

    @contextlib.contextmanager
    def _hook(output_dir, device_ids):
        import jax

        jax.devices()
        if device_ids:
            ids = (ctypes.c_int64 * len(device_ids))(*device_ids)
            rc = lib.axon_start_nrt_profile(ids, len(device_ids))
        else:
            rc = lib.axon_start_nrt_profile(None, 0)
        if rc != 0:
            raise RuntimeError(f"axon_start_nrt_profile rc={rc}")
        try:
            yield
        finally:
            n = lib.axon_stop_nrt_profile(str(output_dir).encode())
            print(f"profile: {n} ntff file(s) written to {output_dir}")

    mod = types.ModuleType("antenv.axon_hooks")
    _holder = {"h": _hook}
    mod.set_axon_ntff_profile_hook = lambda h: _holder.__setitem__("h", h)
    mod.get_axon_ntff_profile_hook = lambda: _holder["h"]
    sys.modules["antenv.axon_hooks"] = mod

    # avoid the S3/Fish artifact upload in the trace post-processing path
    import concourse.bass_utils as bu

    bu.upload_artifacts = lambda tmpdir: str(tmpdir)


def kernel(**inputs):
    from concourse.bass_utils import run_bass_kernel_spmd

    if TRACE:
        _install_ntff_hook()

    x = np.asarray(inputs["x"], np.float32)
    w1 = np.asarray(inputs["w1"], np.float32)
    b1 = np.asarray(inputs["b1"], np.float32)
    w2 = np.asarray(inputs["w2"], np.float32)
    b2 = np.asarray(inputs["b2"], np.float32)
    wg = np.asarray(inputs["wg"], np.float32)
    bg = np.asarray(inputs["bg"], np.float32)

    T = x.shape[0] * x.shape[1]
    xf = x.reshape(T, D)

    # ---- host gating (fp64): logits -> top-2 (jax.lax.top_k tie order:
    # lower index wins -> stable argsort on -logits) -> softmax over top-2.
    logits = xf.astype(np.float64) @ wg.astype(np.float64) + bg.astype(np.float64)
    order = np.argsort(-logits, axis=1, kind="stable")
    top_idx = order[:, :TOP_K]                      # [T, K]
    top_vals = np.take_along_axis(logits, top_idx, axis=1)
    gwts = np.exp(top_vals - top_vals.max(axis=1, keepdims=True))
    gwts = gwts / gwts.sum(axis=1, keepdims=True)   # [T, K]

    # ---- dispatch: sort slots (t, k) by expert; per-expert contiguous runs.
    flat_expert = top_idx.ravel()                   # slot s = t*K + k
    perm = np.argsort(flat_expert, kind="stable")   # slots grouped by expert
    counts = np.bincount(flat_expert, minlength=E)
    cum = np.concatenate([[0], np.cumsum(counts)])
    slot_tok = perm // TOP_K                        # token of each sorted slot
    gates_sorted = gwts.ravel()[perm].astype(np.float32)

    NT = max(512, int(-(-counts.max() // 128)) * 128)
    NTG = NT // 128

    if DT_MODE == "bf16":
        import ml_dtypes
        io_dtype = ml_dtypes.bfloat16
    else:
        io_dtype = np.float32

    xf_io = xf.astype(io_dtype)
    w1_io = w1.astype(io_dtype)
    w2_io = w2.astype(io_dtype)

    in_maps = []
    for e in range(E):
        n = int(counts[e])
        toks = slot_tok[cum[e]:cum[e] + n]
        xt = np.zeros((D, NT), io_dtype)
        xt[:, :n] = xf_io[toks].T
        gate = np.zeros(NT, np.float32)
        gate[:n] = gates_sorted[cum[e]:cum[e] + n]
        in_maps.append({
            "xt": xt,
            "w1t": w1_io[e],
            "w2t": w2_io[e],
            "b1c": np.ascontiguousarray(b1[e].reshape(F // 128, 128).T),
            "gate2": np.ascontiguousarray(gate.reshape(NTG, 128).T),
        })

    nc = _build_program(NT, DT_MODE)
    res = run_bass_kernel_spmd(nc, in_maps, list(range(N_CORES)), trace=TRACE)
    if TRACE and res.exec_time_ns is not None:
        print(f"HW exec time: {res.exec_time_ns} ns")

    # ---- unshard: scatter slots back, sum the K slots per token, add b2 term.
    out_slots = np.zeros((T * TOP_K, D), np.float32)
    for e in range(E):
        n = int(counts[e])
        out_slots[perm[cum[e]:cum[e] + n]] = res.results[e]["y"][:n]
    out = out_slots.reshape(T, TOP_K, D).sum(axis=1)

    # combine @ b2 (gate-weighted expert output biases)
    combine = np.zeros((T, E), np.float32)
    np.put_along_axis(combine, top_idx, gwts.astype(np.float32), axis=1)
    out += combine @ b2

    return out.reshape(B, S, D).astype(np.float32)


# revision 8
# speedup vs baseline: 1.0235x; 1.0084x over previous
"""MoE layer (E=8 experts, top-2 routing) on 8 Trainium2 NeuronCores.

Strategy: expert-parallel. The host computes the (tiny) gating network in
fp64 -- logits = x @ wg + bg, top-2, softmax -- and dispatches each token to
the cores owning its two selected experts (the "all-to-all dispatch tokens
by routing decision" sharding). Each core runs its expert's FFN
    y = relu(x_e @ w1[e] + b1[e]) @ w2[e]
over its gathered tokens (padded to a common static NT), scales rows by the
gate weight on-device, and the host scatter-adds the two slots per token
back together (plus the combine@b2 bias term).

Hardcoded problem shape: x [4,4096,512], w1 [8,512,1024], w2 [8,1024,512],
wg [512,8], top_k=2.
"""

import os
import numpy as np

B, S, D, F, E = 4, 4096, 512, 1024, 8
TOP_K = 2
N_CORES = 8

# matmul operand dtype mode: "bf16" | "f32" | "f32r"
DT_MODE = os.environ.get("MOE_DT", "bf16")
TRACE = os.environ.get("MOE_TRACE", "0") == "1"


def _build_program(NT, mode):
    from concourse import bacc, tile, mybir
    from concourse.tile import add_dep_helper

    dt = mybir.dt
    DT = {"bf16": dt.bfloat16, "f32": dt.float32, "f32r": dt.float32r}[mode]

    nc = bacc.Bacc("TRN2", target_bir_lowering=False, debug=False)

    xt_d = nc.dram_tensor("xt", [D, NT], DT, kind="ExternalInput").ap()
    w1_d = nc.dram_tensor("w1t", [D, F], DT, kind="ExternalInput").ap()
    w2_d = nc.dram_tensor("w2t", [F, D], DT, kind="ExternalInput").ap()
    b1_d = nc.dram_tensor("b1c", [128, F // 128], dt.float32, kind="ExternalInput").ap()
    g_d = nc.dram_tensor("gate2", [128, NT // 128], dt.float32, kind="ExternalInput").ap()
    y_d = nc.dram_tensor("y", [NT, D], dt.float32, kind="ExternalOutput").ap()

    KD = D // 128   # 4 contraction blocks for mm1
    FB = F // 128   # 8 F blocks (partition blocks of h / contraction blocks of mm2)
    # chunk sizes along the token (moving) axis; remainder chunk last
    chunks = [512] * (NT // 512)
    if NT % 512:
        chunks.append(NT % 512)

    with tile.TileContext(nc) as tc:
        with (
            tc.tile_pool(name="w", bufs=1) as wpool,
            tc.tile_pool(name="x", bufs=3) as xpool,
            tc.tile_pool(name="h", bufs=2) as hpool,
            tc.tile_pool(name="o", bufs=4) as opool,
            tc.tile_pool(name="ps1", bufs=4, space="PSUM") as ps1,
            tc.tile_pool(name="ps2", bufs=4, space="PSUM") as ps2,
        ):
    # startup DMA schedule: the PE can start chunk 0 after w1+x0 (3 MB) and
    # needs w2 ~7us later. Split loads in half so all 16 DMA queues engage
    # (per-queue rate is ~20 GB/s), and order later loads behind the first
    # slice of x0 so they don't bandwidth-share with the critical first MBs.
            w1_sb = wpool.tile([128, KD, F], DT)
            w1_dmas = []
            for kc in range(KD):
                for hh in range(2):
                    w1_dmas.append(nc.sync.dma_start(
                        out=w1_sb[:, kc, hh * (F // 2):(hh + 1) * (F // 2)],
                        in_=w1_d[kc * 128:(kc + 1) * 128, hh * (F // 2):(hh + 1) * (F // 2)],
                    ))

            x_tiles = {}
            x_dmas = {}

            def load_x(t):
                cs = chunks[t]
                off = 512 * t
                x_sb = xpool.tile([128, KD, cs], DT, tag="x")
                dmas = []
                nsplit = 2 if cs >= 256 else 1
                for kc in range(KD):
                    for hh in range(nsplit):
                        c0, c1 = hh * (cs // nsplit), (hh + 1) * (cs // nsplit)
                        dmas.append(nc.sync.dma_start(
                            out=x_sb[:, kc, c0:c1],
                            in_=xt_d[kc * 128:(kc + 1) * 128, off + c0:off + c1],
                        ))
                x_tiles[t] = x_sb
                x_dmas[t] = dmas

            load_x(0)

            def after(prev, i, why):
                add_dep_helper(prev.ins, i.ins, sync=True, reason=why)

            # phase B (after first x0 slice lands): w2 split 16 ways + tiny
            # b1/gate, so w2 is resident by the time mm2 of chunk 0 starts.
            gate_dep = x_dmas[0][0]
            w2_sb = wpool.tile([128, FB, D], DT)
            w2_dmas = []
            for fb in range(FB):
                for hh in range(2):
                    i = nc.sync.dma_start(
                        out=w2_sb[:, fb, hh * (D // 2):(hh + 1) * (D // 2)],
                        in_=w2_d[fb * 128:(fb + 1) * 128, hh * (D // 2):(hh + 1) * (D // 2)],
                    )
                    w2_dmas.append(i)
                    after(gate_dep, i, "dma order: w2 after first x0 slice")
            b1_sb = wpool.tile([128, FB], dt.float32)
            after(gate_dep, nc.sync.dma_start(out=b1_sb[:], in_=b1_d[:]), "dma order")
            g_sb = wpool.tile([128, NT // 128], dt.float32)
            after(gate_dep, nc.sync.dma_start(out=g_sb[:], in_=g_d[:]), "dma order")

            for t in range(len(chunks)):
                cs = chunks[t]
                off = 512 * t
                if t + 1 < len(chunks):
                    load_x(t + 1)
                x_sb = x_tiles.pop(t)
                h_sb = hpool.tile([128, FB, cs], DT, tag="h")
                for fb in range(FB):
                    p = ps1.tile([128, cs], dt.float32, tag="ps1")
                    for kc in range(KD):
                        nc.tensor.matmul(
                            p[:],
                            w1_sb[:, kc, fb * 128:(fb + 1) * 128],
                            x_sb[:, kc, :],
                            start=(kc == 0),
                            stop=(kc == KD - 1),
                        )
                    nc.scalar.activation(
                        h_sb[:, fb, :],
                        p[:],
                        mybir.ActivationFunctionType.Relu,
                        bias=b1_sb[:, fb:fb + 1],
                        scale=1.0,
                    )
                for tb in range(cs // 128):
                    p2 = ps2.tile([128, 512], dt.float32, tag="ps2")
                    for fb in range(FB):
                        nc.tensor.matmul(
                            p2[:],
                            h_sb[:, fb, tb * 128:(tb + 1) * 128],
                            w2_sb[:, fb, :],
                            start=(fb == 0),
                            stop=(fb == FB - 1),
                        )
                    o_sb = opool.tile([128, 512], dt.float32, tag="o")
                    nc.vector.tensor_scalar_mul(
                        o_sb[:], p2[:], g_sb[:, t * 4 + tb:t * 4 + tb + 1]
                    )
                    nc.sync.dma_start(
                        out=y_d[off + tb * 128:off + (tb + 1) * 128, :],
                        in_=o_sb[:],
                    )
    nc.compile()
    return nc


def _install_ntff_hook():
    """Register the axon NTFF profiling hook that run_bass_kernel_spmd
    (trace=True) looks for under antenv.axon_hooks; this container's antenv
    lacks that module, so recreate it via ctypes against libaxon_pjrt.so."""
    import sys, types, ctypes, contextlib

    if "antenv.axon_hooks" in sys.modules:
        return
    try:
        lib = ctypes.CDLL("/opt/axon/libaxon_pjrt.so")
    except OSError:
        return
    if not hasattr(lib, "axon_start_nrt_profile"):
        return
    lib.axon_start_nrt_profile.argtypes = [ctypes.POINTER(ctypes.c_int64), ctypes.c_size_t]
    lib.axon_start_nrt_profile.restype = ctypes.c_int64
    lib.axon_stop_nrt_profile.argtypes = [ctypes.c_char_p]
    lib.axon_stop_nrt_profile.restype = ctypes.c_int64

    @contextlib.contextmanager
    def _hook(output_dir, device_ids):
        import jax

        jax.devices()
        if device_ids:
            ids = (ctypes.c_int64 * len(device_ids))(*device_ids)
            rc = lib.axon_start_nrt_profile(ids, len(device_ids))
        else:
            rc = lib.axon_start_nrt_profile(None, 0)
        if rc != 0:
            raise RuntimeError(f"axon_start_nrt_profile rc={rc}")
        try:
            yield
        finally:
            n = lib.axon_stop_nrt_profile(str(output_dir).encode())
            print(f"profile: {n} ntff file(s) written to {output_dir}")

    mod = types.ModuleType("antenv.axon_hooks")
    _holder = {"h": _hook}
    mod.set_axon_ntff_profile_hook = lambda h: _holder.__setitem__("h", h)
    mod.get_axon_ntff_profile_hook = lambda: _holder["h"]
    sys.modules["antenv.axon_hooks"] = mod

    # avoid the S3/Fish artifact upload in the trace post-processing path
    import concourse.bass_utils as bu

    bu.upload_artifacts = lambda tmpdir: str(tmpdir)


def kernel(**inputs):
    from concourse.bass_utils import run_bass_kernel_spmd

    if TRACE:
        _install_ntff_hook()

    x = np.asarray(inputs["x"], np.float32)
    w1 = np.asarray(inputs["w1"], np.float32)
    b1 = np.asarray(inputs["b1"], np.float32)
    w2 = np.asarray(inputs["w2"], np.float32)
    b2 = np.asarray(inputs["b2"], np.float32)
    wg = np.asarray(inputs["wg"], np.float32)
    bg = np.asarray(inputs["bg"], np.float32)

    T = x.shape[0] * x.shape[1]
    xf = x.reshape(T, D)

    # ---- host gating (fp64): logits -> top-2 (jax.lax.top_k tie order:
    # lower index wins -> stable argsort on -logits) -> softmax over top-2.
    logits = xf.astype(np.float64) @ wg.astype(np.float64) + bg.astype(np.float64)
    order = np.argsort(-logits, axis=1, kind="stable")
    top_idx = order[:, :TOP_K]                      # [T, K]
    top_vals = np.take_along_axis(logits, top_idx, axis=1)
    gwts = np.exp(top_vals - top_vals.max(axis=1, keepdims=True))
    gwts = gwts / gwts.sum(axis=1, keepdims=True)   # [T, K]

    # ---- dispatch: sort slots (t, k) by expert; per-expert contiguous runs.
    flat_expert = top_idx.ravel()                   # slot s = t*K + k
    perm = np.argsort(flat_expert, kind="stable")   # slots grouped by expert
    counts = np.bincount(flat_expert, minlength=E)
    cum = np.concatenate([[0], np.cumsum(counts)])
    slot_tok = perm // TOP_K                        # token of each sorted slot
    gates_sorted = gwts.ravel()[perm].astype(np.float32)

    NT = max(512, int(-(-counts.max() // 128)) * 128)
    NTG = NT // 128

    if DT_MODE == "bf16":
        import ml_dtypes
        io_dtype = ml_dtypes.bfloat16
    else:
        io_dtype = np.float32

    xf_io = xf.astype(io_dtype)
    w1_io = w1.astype(io_dtype)
    w2_io = w2.astype(io_dtype)

    in_maps = []
    for e in range(E):
        n = int(counts[e])
        toks = slot_tok[cum[e]:cum[e] + n]
        xt = np.zeros((D, NT), io_dtype)
        xt[:, :n] = xf_io[toks].T
        gate = np.zeros(NT, np.float32)
        gate[:n] = gates_sorted[cum[e]:cum[e] + n]
        in_maps.append({
            "xt": xt,
            "w1t": w1_io[e],
            "w2t": w2_io[e],
            "b1c": np.ascontiguousarray(b1[e].reshape(F // 128, 128).T),
            "gate2": np.ascontiguousarray(gate.reshape(NTG, 128).T),
        })

    nc = _build_program(NT, DT_MODE)
    res = run_bass_kernel_spmd(nc, in_maps, list(range(N_CORES)), trace=TRACE)
    if TRACE and res.exec_time_ns is not None:
        print(f"HW exec time: {res.exec_time_ns} ns")

    # ---- unshard: scatter slots back, sum the K slots per token, add b2 term.
    out_slots = np.zeros((T * TOP_K, D), np.float32)
    for e in range(E):
        n = int(counts[e])
        out_slots[perm[cum[e]:cum[e] + n]] = res.results[e]["y"][:n]
    out = out_slots.reshape(T, TOP_K, D).sum(axis=1)

    # combine @ b2 (gate-weighted expert output biases)
    combine = np.zeros((T, E), np.float32)
    np.put_along_axis(combine, top_idx, gwts.astype(np.float32), axis=1)
    out += combine @ b2

    return out.reshape(B, S, D).astype(np.float32)
